# revision 22
# baseline (speedup 1.0000x reference)
"""Fused transformer block (RMSNorm + qk-norm attention + MLP) for TRN2, 8 cores.

Sharding: 8 cores = (4 batches) x (2 query-halves). Each core gets its batch's
full sequence with rows rotated so its query half is rows 0..1023 (attention is
permutation-invariant over keys, so K/V row order doesn't matter). No
collectives needed; each core produces a disjoint [1024, 768] output slice.

v2 changes vs baseline (768us):
  - ln1 normalization is never materialized: qk-rmsnorm is scale-invariant, so
    Q/K project the RAW latents (ln1_scale folded into weights); V applies the
    per-row 1/rms as a per-partition scalar in natural layout.
  - All DRAM-transpose round trips (xT, KT, QT, x2T) are chunked at 512 rows
    and pipelined with their producer loops -> no phase-boundary PE bubbles
    (the bubbles let HAM re-throttle the PE to 1.2 GHz; the baseline ran the
    entire 443us attention phase at half clock).
  - Softmax exp is split between ScalarE (table exp) and VectorE (Schraudolph
    bit-trick exp: bf16 bits = int16(l * 128*log2e + 16252.07), one
    tensor_scalar with f32->i16 convert-on-write). Keeps both engines busy and
    PE gap-free in attention.
  - Softmax denominators use reciprocal_approx_fast (custom DVE op, ~1 cyc/elem)
    instead of the iterative divide (~6 cyc/elem on a single partition).
  - MLP weights prefetch during attention / out-proj phases.
"""

import numpy as np
from contextlib import ExitStack

import concourse.bass as bass
import concourse.tile as tile
from concourse import bacc, mybir
from concourse.bass_utils import run_bass_kernel_spmd

F32 = mybir.dt.float32
BF16 = mybir.dt.bfloat16
I16 = mybir.dt.int16
AF = mybir.ActivationFunctionType
OP = mybir.AluOpType

B, S, D, H, HD, MLP = 4, 2048, 768, 12, 64, 3072
SQ = S // 2            # query rows per core
NT_S = S // 128        # 16 sequence tiles
NT_Q = SQ // 128       # 8 query tiles
NT_D = D // 128        # 6 model-dim tiles
NT_M = MLP // 128      # 24 mlp-dim tiles
EPS = 1e-6
VW = HD + 1            # V width incl. ones column
CH = 512               # transpose chunk (rows)
NC_S = S // CH         # 4 chunks over full sequence
NC_Q = SQ // CH        # 2 chunks over query rows

# Schraudolph exp at bf16 scale: bits16 = trunc(l * 2^7*log2e + B_EXP);
# B_EXP = 127*128 - 4.43 (minimax bias) + 0.5 (trunc compensation).
A_EXP = 1.4426950408889634 * 128.0
B_EXP = 127.0 * 128.0 - 4.43 + 0.5


DVE_EXP = True
FAST_RECIP = True


def _dve_exp(t, e):
    # which exp tiles go to VectorE (rest on ScalarE). Skip the first two t of
    # each head-pair (DVE still draining the previous pair's evac/divides).
    return DVE_EXP and t >= 2 and (2 * t + e) % 6 < 2


def _chunks(n):
    out, ofs = [], 0
    while ofs < n:
        c = min(512, n - ofs)
        out.append((ofs, c))
        ofs += c
    return out


def build_nc(sim_compat=False):
    nc = bacc.Bacc("TRN2", target_bir_lowering=False, debug=False, num_devices=8)

    lat = nc.dram_tensor("lat", [S, D], F32, kind="ExternalInput").ap()
    wq = nc.dram_tensor("wq", [D, D], BF16, kind="ExternalInput").ap()
    wk = nc.dram_tensor("wk", [D, D], BF16, kind="ExternalInput").ap()
    wv = nc.dram_tensor("wv", [D, D], BF16, kind="ExternalInput").ap()
    wo = nc.dram_tensor("wo", [D, D], BF16, kind="ExternalInput").ap()
    wi = nc.dram_tensor("wi", [D, MLP], BF16, kind="ExternalInput").ap()
    wom = nc.dram_tensor("wom", [MLP, D], BF16, kind="ExternalInput").ap()
    kqsc = nc.dram_tensor("kqsc", [128, 1], F32, kind="ExternalInput").ap()
    out = nc.dram_tensor("out", [SQ, D], F32, kind="ExternalOutput").ap()

    with tile.TileContext(nc) as tc, ExitStack() as top:
        def ptile(pool, shape, dtype, name):
            return pool.tile(shape, dtype, name=name, tag=name)

        p_const = top.enter_context(tc.tile_pool(name="p_const", bufs=1))
        p_x2 = top.enter_context(tc.tile_pool(name="p_x2", bufs=1))
        p_oT = tc.alloc_tile_pool(name="p_oT", bufs=1)
        p_att = tc.alloc_tile_pool(name="p_att", bufs=1)

        # ---- persistent tiles ----
        Vaug = ptile(p_att, [128, NT_S * H * VW], BF16, name="Vaug")
        oT = ptile(p_oT, [128, NT_D * SQ], BF16, name="oT")
        kqsc_t = ptile(p_const, [128, 1], F32, name="kqsc_t")
        onesF = ptile(p_const, [128, 64], F32, name="onesF")
        eps_t = ptile(p_const, [128, 1], F32, name="eps_t")
        rs_all = ptile(p_const, [128, NT_S], F32, name="rs_all")
        KT = [ptile(p_att, [128, S], BF16, name=f"KT{d}") for d in range(NT_D)]
        QT = [ptile(p_att, [128, SQ], BF16, name=f"QT{d}") for d in range(NT_D)]
        x2 = [ptile(p_x2, [128, D], F32, name=f"x2_{q}") for q in range(NT_Q)]
        x2T = [ptile(p_x2, [128, SQ], BF16, name=f"x2T{d}") for d in range(NT_D)]

        nc.sync.dma_start(kqsc_t[:], kqsc[:])
        nc.vector.memset(eps_t[:], EPS)
        nc.vector.memset(onesF[:], 1.0)
        vview = Vaug[:].rearrange("p (s h k) -> p s h k", s=NT_S, h=H)
        nc.vector.memset(vview[:, :, :, HD:VW], 1.0)

        dram = top.enter_context(tc.tile_pool(name="dram", bufs=1, space="DRAM"))
        xh_d = dram.tile([S, D], BF16, name="xh_d")
        kh_d = dram.tile([S, D], BF16, name="kh_d")
        qh_d = dram.tile([SQ, D], BF16, name="qh_d")
        x2h_d = dram.tile([SQ, D], BF16, name="x2h_d")

        # QKV weights: prefetch on the scalar HWDGE queue so the sync queue
        # stays free for the Phase A latent loads.
        p_wqkv = tc.alloc_tile_pool(name="p_wqkv", bufs=1, side="right")
        wq_sb = [ptile(p_wqkv, [128, D], BF16, name=f"wq_sb{d}") for d in range(NT_D)]
        wk_sb = [ptile(p_wqkv, [128, D], BF16, name=f"wk_sb{d}") for d in range(NT_D)]
        wv_sb = [ptile(p_wqkv, [128, D], BF16, name=f"wv_sb{d}") for d in range(NT_D)]
        for w_sb, w_d in ((wk_sb, wk), (wv_sb, wv), (wq_sb, wq)):
            for d in range(NT_D):
                nc.scalar.dma_start(w_sb[d][:], w_d[d * 128:(d + 1) * 128, :])

        # =============== Phase A: ln1 stats + raw-x bf16 + x^T ===============
        # qk-rmsnorm is invariant to the per-row ln1 scaling, so Q/K project
        # the raw latents; only V needs the 1/rms factor (applied per-partition
        # in natural layout in Phase B). x^T tiles hold RAW bf16 latents.
        p_xT = tc.alloc_tile_pool(name="p_xT", bufs=1, side="right")
        xT = [ptile(p_xT, [128, S], BF16, name=f"xT{d}") for d in range(NT_D)]
        with ExitStack() as ctx:
            io = ctx.enter_context(tc.tile_pool(name="a_io", bufs=8))
            st_p = ctx.enter_context(tc.tile_pool(name="a_stats", bufs=8))
            scr = ctx.enter_context(tc.tile_pool(name="a_scr", bufs=4))
            # all loads issued up front: the chunk transposes interleaved below
            # would otherwise head-of-line block later loads on the sync queue
            lts = []
            for t in range(NT_S):
                lt = io.tile([128, D], F32, name="lt")
                nc.sync.dma_start(lt[:], lat[t * 128:(t + 1) * 128, :])
                lts.append(lt)
            for t in range(NT_S):
                lt = lts[t]
                xh = scr.tile([128, D], BF16, name="xh")
                nc.vector.tensor_copy(xh[:], lt[:])
                nc.gpsimd.dma_start(xh_d[t * 128:(t + 1) * 128, :], xh[:])
                sq = scr.tile([128, D], F32, name="sq")
                ssq = st_p.tile([128, 1], F32, name="ssq")
                nc.scalar.activation(sq[:], lt[:], AF.Square, accum_out=ssq[:])
                srt = st_p.tile([128, 1], F32, name="srt")
                nc.scalar.activation(srt[:], ssq[:], AF.Sqrt, bias=eps_t[:], scale=1.0 / D)
                nc.vector.reciprocal(rs_all[:, t:t + 1], srt[:])
                if t % 4 == 3:  # chunk of 512 rows complete -> transpose it
                    c = t // 4
                    for d in range(NT_D):
                        nc.sync.dma_start_transpose(
                            xT[d][:, c * CH:(c + 1) * CH],
                            xh_d[c * CH:(c + 1) * CH, d * 128:(d + 1) * 128])

        # =============== Phase B: Q/K/V projections + qk-norm ===============
        with ExitStack() as ctx:
            ps = ctx.enter_context(tc.tile_pool(name="b_ps", bufs=4, space="PSUM"))
            scr = ctx.enter_context(tc.tile_pool(name="b_scr", bufs=3))
            st_p = ctx.enter_context(tc.tile_pool(name="b_stats", bufs=6))
            natp = ctx.enter_context(tc.tile_pool(name="b_nat", bufs=3))

            def proj(t, w_sb):
                p = ps.tile([128, D], F32, name="p_proj")
                for d in range(NT_D):
                    lhsT = xT[d][:, t * 128:(t + 1) * 128]
                    for ofs, n in _chunks(D):
                        nc.tensor.matmul(
                            p[:, ofs:ofs + n], lhsT, w_sb[d][:, ofs:ofs + n],
                            start=(d == 0), stop=(d == NT_D - 1))
                return p

            def qknorm(p, dst_dram, t):
                sq = scr.tile([128, D], F32, name="sq_b")
                nc.scalar.activation(sq[:], p[:], AF.Square)
                ss = st_p.tile([128, H], F32, name="ss_b")
                nc.vector.tensor_reduce(
                    ss[:], sq[:].rearrange("p (h k) -> p h k", h=H),
                    axis=mybir.AxisListType.X, op=OP.add)
                srt = st_p.tile([128, H], F32, name="srt_b")
                nc.scalar.activation(srt[:], ss[:], AF.Sqrt, bias=eps_t[:], scale=1.0 / HD)
                rs = st_p.tile([128, H], F32, name="rs_b")
                nc.vector.reciprocal(rs[:], srt[:])
                nat = natp.tile([128, D], BF16, name="nat_b")
                rs_view = rs[:].rearrange("p (h o) -> p h o", o=1).broadcast_to([128, H, HD])
                nc.vector.tensor_tensor(
                    out=nat[:].rearrange("p (h k) -> p h k", h=H),
                    in0=p[:].rearrange("p (h k) -> p h k", h=H),
                    in1=rs_view, op=OP.mult)
                nc.gpsimd.dma_start(dst_dram[t * 128:(t + 1) * 128, :], nat[:])

            for t in range(NT_S):
                pk = proj(t, wk_sb)
                qknorm(pk, kh_d, t)
                pv = proj(t, wv_sb)
                nc.vector.tensor_scalar_mul(
                    vview[:, t, :, 0:HD],
                    pv[:].rearrange("p (h k) -> p h k", h=H),
                    rs_all[:, t:t + 1])
                if t < NT_Q:
                    pq = proj(t, wq_sb)
                    qknorm(pq, qh_d, t)
                if t % 4 == 3:  # pipeline K^T (and Q^T) transposes per 512-row chunk
                    c = t // 4
                    for d in range(NT_D):
                        nc.sync.dma_start_transpose(
                            KT[d][:, c * CH:(c + 1) * CH],
                            kh_d[c * CH:(c + 1) * CH, d * 128:(d + 1) * 128])
                        if c < NC_Q:
                            # kqsc (qk-norm scales / sqrt(hd)) folded into Q^T
                            # only: logits contract q*k over hd, so one side
                            # suffices (12 scale ops instead of 24 on K^T).
                            nc.sync.dma_start_transpose(
                                QT[d][:, c * CH:(c + 1) * CH],
                                qh_d[c * CH:(c + 1) * CH, d * 128:(d + 1) * 128])
            # QT kqsc scales batched here: the transposes are complete, so
            # these don't head-of-line block the DVE queue mid-loop.
            for d in range(NT_D):
                nc.vector.tensor_scalar_mul(QT[d][:], QT[d][:], kqsc_t[:])
        p_xT.release()
        p_wqkv.release()

        # MLP1 + attn-out weights: prefetch during attention (sync queue idle)
        p_wi = tc.alloc_tile_pool(name="p_wi", bufs=1, side="right")
        wi_sb = [ptile(p_wi, [128, MLP], BF16, name=f"wi_sb{d}") for d in range(NT_D)]
        p_wo = tc.alloc_tile_pool(name="p_wo", bufs=1, side="right")
        wo_sb = [ptile(p_wo, [128, D], BF16, name=f"wo_sb{d}") for d in range(NT_D)]
        for d in range(NT_D):
            nc.sync.dma_start(wo_sb[d][:], wo[d * 128:(d + 1) * 128, :])
            nc.sync.dma_start(wi_sb[d][:], wi[d * 128:(d + 1) * 128, :])

        # =============== Phase C: attention ===============
        with ExitStack() as ctx:
            psL = ctx.enter_context(tc.tile_pool(name="c_psL", bufs=2, space="PSUM"))
            psO = ctx.enter_context(tc.tile_pool(name="c_psO", bufs=2, space="PSUM"))
            pp = ctx.enter_context(tc.tile_pool(name="c_p", bufs=6))
            oup = ctx.enter_context(tc.tile_pool(name="c_oU", bufs=5))

            def divide_head(h, oU_h, den_h, row):
                # broadcast 1/denom (at partition `row` of den_h) across 64
                # partitions via ones outer product, then multiply.
                dt, base = h // 2, (h % 2) * 64
                b_ps = psL.tile([64, SQ], F32, name="b_ps", tag="l_ps")
                for ofs, n in _chunks(SQ):
                    nc.tensor.matmul(b_ps[:, ofs:ofs + n], onesF[row:row + 1, :],
                                     den_h[row:row + 1, ofs:ofs + n],
                                     start=True, stop=True)
                nc.vector.scalar_tensor_tensor(
                    oT[base:base + 64, dt * SQ:(dt + 1) * SQ],
                    b_ps[:], 1.0, oU_h[0:HD, :], op0=OP.bypass, op1=OP.mult)

            def attn_v(hp, t, o_ps, p_t):
                for e in range(2):
                    vofs = t * H * VW + (2 * hp + e) * VW
                    for ofs, n in _chunks(SQ):
                        nc.tensor.matmul(
                            o_ps[e][:, ofs:ofs + n],
                            Vaug[:, vofs:vofs + VW],
                            p_t[e][:].bitcast(BF16)[:, ofs:ofs + n],
                            start=(t == 0), stop=(t == NT_S - 1))

            pending = []
            for hp in range(H // 2):
                dt = hp
                o_ps = [psO.tile([VW, SQ], F32, name=f"o_ps{e}", tag="o_ps")
                        for e in range(2)]
                prev = None  # software pipeline: attnV runs one t behind the
                # logits/exp of t, so the exp has a full extra unit of slack
                # and the PE stream stays gap-free (keeps HAM at full clock).
                for t in range(NT_S):
                    l_ps = [psL.tile([128, SQ], F32, name=f"l_ps{e}", tag="l_ps")
                            for e in range(2)]
                    for e in range(2):  # head 2*hp+e at PE row group 64*e
                        base = 64 * e
                        lhsT = KT[dt][base:base + 64, t * 128:(t + 1) * 128]
                        for j in range(NC_Q):
                            nc.tensor.matmul(
                                l_ps[e][:, j * CH:(j + 1) * CH], lhsT,
                                QT[dt][base:base + 64, j * CH:(j + 1) * CH],
                                start=True, stop=True)
                    # attnV of t-1 emitted before the exps of t so the PE
                    # queue keeps the two logits pairs adjacent (row-group
                    # concurrency) and attnV consumes the already-finished exp.
                    if prev is not None:
                        attn_v(hp, prev[0], o_ps, prev[1])
                    p_t = [None, None]
                    for e in range(2):
                        p_t[e] = pp.tile([128, SQ], I16, name=f"p_t{e}", tag="p_t")
                        if _dve_exp(t, e):
                            nc.vector.tensor_scalar(
                                p_t[e][:], l_ps[e][:], A_EXP, B_EXP,
                                op0=OP.mult, op1=OP.add)
                        else:
                            nc.scalar.activation(
                                p_t[e][:].bitcast(BF16), l_ps[e][:], AF.Exp)
                    prev = (t, p_t)
                    if t == 4:
                        for h_prev, oU_prev, den_prev, row in pending:
                            divide_head(h_prev, oU_prev, den_prev, row)
                        pending = []
                attn_v(hp, prev[0], o_ps, prev[1])
                # denominator rows of both heads -> partitions 0/32 of one
                # scratch (custom-DVE ops only work at partition offset 0),
                # one batched reciprocal for the pair.
                den_t = oup.tile([33, SQ], F32, name="den", tag="den")
                oUs = []
                for e in range(2):
                    oU_h = oup.tile([VW, SQ], F32, name="oU", tag="oU")
                    nc.vector.tensor_copy(oU_h[:], o_ps[e][:])
                    nc.sync.dma_start(den_t[32 * e:32 * e + 1, :],
                                      oU_h[VW - 1:VW, :])
                    oUs.append(oU_h)
                if FAST_RECIP:
                    nc.vector.reciprocal_approx_fast(den_t[:], den_t[:])
                else:
                    nc.vector.reciprocal(den_t[0:1, :], den_t[0:1, :])
                    nc.vector.reciprocal(den_t[32:33, :], den_t[32:33, :])
                for e in range(2):
                    pending.append((2 * hp + e, oUs[e], den_t, 32 * e))
            for h_prev, oU_prev, den_prev, row in pending:
                divide_head(h_prev, oU_prev, den_prev, row)
        p_att.release()

        # MLP2 weights: prefetch on the scalar HWDGE queue during Phase D
        # (the sync queue carries the lat loads + x2T transposes there).
        p_wom = tc.alloc_tile_pool(name="p_wom", bufs=1, side="right")
        wom_sb = [ptile(p_wom, [128, D], BF16, name=f"wom_sb{m}") for m in range(NT_M)]
        for m in range(NT_M):
            nc.scalar.dma_start(wom_sb[m][:], wom[m * 128:(m + 1) * 128, :])

        # =============== Phase D: out-proj + residual + ln2 ===============
        with ExitStack() as ctx:
            ps = ctx.enter_context(tc.tile_pool(name="d_ps", bufs=2, space="PSUM"))
            io = ctx.enter_context(tc.tile_pool(name="d_io", bufs=NT_Q))
            scr = ctx.enter_context(tc.tile_pool(name="d_scr", bufs=3))
            st_p = ctx.enter_context(tc.tile_pool(name="d_stats", bufs=4))

            lts = []
            for q in range(NT_Q):
                lt = io.tile([128, D], F32, name="lt_d")
                nc.sync.dma_start(lt[:], lat[q * 128:(q + 1) * 128, :])
                lts.append(lt)
            for q in range(NT_Q):
                p = ps.tile([128, D], F32, name="p_oproj")
                for d in range(NT_D):
                    for ofs, n in _chunks(D):
                        nc.tensor.matmul(
                            p[:, ofs:ofs + n],
                            oT[:, d * SQ + q * 128: d * SQ + (q + 1) * 128],
                            wo_sb[d][:, ofs:ofs + n],
                            start=(d == 0), stop=(d == NT_D - 1))
                nc.vector.tensor_tensor(out=x2[q][:], in0=p[:], in1=lts[q][:], op=OP.add)
                sq = scr.tile([128, D], F32, name="sq_d")
                ssq = st_p.tile([128, 1], F32, name="ssq_d")
                nc.scalar.activation(sq[:], x2[q][:], AF.Square, accum_out=ssq[:])
                srt = st_p.tile([128, 1], F32, name="srt_d")
                nc.scalar.activation(srt[:], ssq[:], AF.Sqrt, bias=eps_t[:], scale=1.0 / D)
                rs = st_p.tile([128, 1], F32, name="rs_d")
                nc.vector.reciprocal(rs[:], srt[:])
                xh2 = scr.tile([128, D], BF16, name="xh2")
                nc.vector.tensor_scalar_mul(xh2[:], x2[q][:], rs[:])
                nc.gpsimd.dma_start(x2h_d[q * 128:(q + 1) * 128, :], xh2[:])
                if q % 4 == 3:  # pipeline x2^T transposes per 512-row chunk
                    c = q // 4
                    for d in range(NT_D):
                        nc.sync.dma_start_transpose(
                            x2T[d][:, c * CH:(c + 1) * CH],
                            x2h_d[c * CH:(c + 1) * CH, d * 128:(d + 1) * 128])
        p_oT.release()

        # =============== Phase E: MLP ===============
        p_hT = tc.alloc_tile_pool(name="p_hT", bufs=1, side="right")
        hT = ptile(p_hT, [128, NT_M * SQ], BF16, name="hT")
        with ExitStack() as ctx:
            ps = ctx.enter_context(tc.tile_pool(name="e_ps", bufs=1, space="PSUM"))
            iop = ctx.enter_context(tc.tile_pool(name="e_io", bufs=3))

            for m in range(NT_M):
                p = ps.tile([128, SQ], F32, name="p_mlp1", bufs=2)
                # j-outer: the j=0 half only needs x2T chunk 0 (first half of
                # Phase D) -> PE starts MLP1 while D finishes.
                for j in range(NC_Q):
                    for d in range(NT_D):
                        nc.tensor.matmul(
                            p[:, j * CH:(j + 1) * CH],
                            wi_sb[d][:, m * 128:(m + 1) * 128],
                            x2T[d][:, j * CH:(j + 1) * CH],
                            start=(d == 0), stop=(d == NT_D - 1))
                if not sim_compat:
                    nc.scalar.activation(hT[:, m * SQ:(m + 1) * SQ], p[:],
                                         AF.Gelu_apprx_tanh)
                else:
                    xsq = iop.tile([128, SQ], F32, name="g_xsq", bufs=1)
                    nc.vector.tensor_tensor(out=xsq[:], in0=p[:], in1=p[:], op=OP.mult)
                    w = iop.tile([128, SQ], F32, name="g_w", bufs=1)
                    nc.vector.tensor_scalar(w[:], xsq[:], 0.044715, 1.0,
                                            op0=OP.mult, op1=OP.add)
                    u = iop.tile([128, SQ], F32, name="g_u", bufs=1)
                    nc.vector.tensor_tensor(out=u[:], in0=w[:], in1=p[:], op=OP.mult)
                    th = iop.tile([128, SQ], F32, name="g_th", bufs=1)
                    nc.scalar.activation(th[:], u[:], AF.Tanh, scale=0.7978845608028654)
                    t2 = iop.tile([128, SQ], F32, name="g_t2", bufs=1)
                    nc.vector.scalar_tensor_tensor(t2[:], th[:], 1.0, p[:],
                                                   op0=OP.add, op1=OP.mult)
                    nc.vector.tensor_scalar_mul(hT[:, m * SQ:(m + 1) * SQ], t2[:], 0.5)

            for q in range(NT_Q):
                p = ps.tile([128, D], F32, name="p_mlp2", bufs=2)
                for m in range(NT_M):
                    for ofs, n in _chunks(D):
                        nc.tensor.matmul(
                            p[:, ofs:ofs + n],
                            hT[:, m * SQ + q * 128: m * SQ + (q + 1) * 128],
                            wom_sb[m][:, ofs:ofs + n],
                            start=(m == 0), stop=(m == NT_M - 1))
                ot = iop.tile([128, D], F32, name="ot_e")
                nc.vector.tensor_tensor(out=ot[:], in0=p[:], in1=x2[q][:], op=OP.add)
                nc.sync.dma_start(out[q * 128:(q + 1) * 128, :], ot[:])
        p_hT.release()
        p_wom.release()
        p_wo.release()
        p_wi.release()

    nc.compile()
    return nc


def make_in_maps(latents, ln1_scale, wq, wk, wv, q_norm_scale, k_norm_scale,
                 wo_attn, ln2_scale, wi, wo_mlp):
    import ml_dtypes
    bf = ml_dtypes.bfloat16
    wq2 = (np.asarray(ln1_scale, np.float64)[:, None]
           * np.asarray(wq, np.float64).reshape(D, D)).astype(bf)
    wk2 = (np.asarray(ln1_scale, np.float64)[:, None]
           * np.asarray(wk, np.float64).reshape(D, D)).astype(bf)
    wv2 = (np.asarray(ln1_scale, np.float64)[:, None]
           * np.asarray(wv, np.float64).reshape(D, D)).astype(bf)
    wo2 = np.asarray(wo_attn, np.float32).reshape(D, D).astype(bf)
    wi2 = (np.asarray(ln2_scale, np.float64)[:, None]
           * np.asarray(wi, np.float64)).astype(bf)
    wom2 = np.asarray(wo_mlp, np.float32).astype(bf)
    kq = (np.tile(np.asarray(q_norm_scale, np.float64)
                  * np.asarray(k_norm_scale, np.float64), 2)
          / np.sqrt(HD)).astype(np.float32)[:, None]
    lat_np = np.asarray(latents, np.float32)
    in_maps = []
    for c in range(8):
        b, half = c // 2, c % 2
        lm = lat_np[b]
        lat_rot = np.concatenate([lm[half * SQ:(half + 1) * SQ],
                                  lm[(1 - half) * SQ:(2 - half) * SQ]], axis=0)
        in_maps.append(dict(lat=np.ascontiguousarray(lat_rot), wq=wq2, wk=wk2,
                            wv=wv2, wo=wo2, wi=wi2, wom=wom2, kqsc=kq))
    return in_maps


_NC_CACHE = None


def kernel(**inputs):
    global _NC_CACHE
    if _NC_CACHE is None:
        _NC_CACHE = build_nc()
    nc = _NC_CACHE
    in_maps = make_in_maps(**inputs)
    res = run_bass_kernel_spmd(nc, in_maps, list(range(8)))
    y = np.empty((B, S, D), np.float32)
    for c in range(8):
        b, half = c // 2, c % 2
        y[b, half * SQ:(half + 1) * SQ] = res.results[c]["out"]
    return y


if __name__ == "__main__":
    import reference
    inputs = {k: np.asarray(v) for k, v in reference.setup_inputs().items()}
    y = kernel(**inputs)
    exp = np.asarray(reference.reference(**reference.setup_inputs()))
    err = np.abs(y - exp).max() / np.abs(exp).max()
    print("Relative error:", err)


# revision 28
# speedup vs baseline: 1.0737x; 1.0737x over previous
"""Fused transformer block (RMSNorm + qk-norm attention + MLP) for TRN2, 8 cores.

Sharding: 8 cores = (4 batches) x (2 query-halves). Each core gets its batch's
full sequence with rows rotated so its query half is rows 0..1023 (attention is
permutation-invariant over keys, so K/V row order doesn't matter). No
collectives needed; each core produces a disjoint [1024, 768] output slice.

v2 changes vs baseline (768us):
  - ln1 normalization is never materialized: qk-rmsnorm is scale-invariant, so
    Q/K project the RAW latents (ln1_scale folded into weights); V applies the
    per-row 1/rms as a per-partition scalar in natural layout.
  - All DRAM-transpose round trips (xT, KT, QT, x2T) are chunked at 512 rows
    and pipelined with their producer loops -> no phase-boundary PE bubbles
    (the bubbles let HAM re-throttle the PE to 1.2 GHz; the baseline ran the
    entire 443us attention phase at half clock).
  - Softmax exp is split between ScalarE (table exp) and VectorE (Schraudolph
    bit-trick exp: bf16 bits = int16(l * 128*log2e + 16252.07), one
    tensor_scalar with f32->i16 convert-on-write). Keeps both engines busy and
    PE gap-free in attention.
  - Softmax denominators use reciprocal_approx_fast (custom DVE op, ~1 cyc/elem)
    instead of the iterative divide (~6 cyc/elem on a single partition).
  - MLP weights prefetch during attention / out-proj phases.
"""

import numpy as np
from contextlib import ExitStack

import concourse.bass as bass
import concourse.tile as tile
from concourse import bacc, mybir
from concourse.bass_utils import run_bass_kernel_spmd

F32 = mybir.dt.float32
BF16 = mybir.dt.bfloat16
I16 = mybir.dt.int16
AF = mybir.ActivationFunctionType
OP = mybir.AluOpType

B, S, D, H, HD, MLP = 4, 2048, 768, 12, 64, 3072
SQ = S // 2            # query rows per core
NT_S = S // 128        # 16 sequence tiles
NT_Q = SQ // 128       # 8 query tiles
NT_D = D // 128        # 6 model-dim tiles
NT_M = MLP // 128      # 24 mlp-dim tiles
EPS = 1e-6
VW = HD + 1            # V width incl. ones column
CH = 512               # transpose chunk (rows)
NC_S = S // CH         # 4 chunks over full sequence
NC_Q = SQ // CH        # 2 chunks over query rows

# Schraudolph exp at bf16 scale: bits16 = trunc(l * 2^7*log2e + B_EXP);
# B_EXP = 127*128 - 4.43 (minimax bias) + 0.5 (trunc compensation).
A_EXP = 1.4426950408889634 * 128.0
B_EXP = 127.0 * 128.0 - 4.43 + 0.5


DVE_EXP = True
FAST_RECIP = True


def _dve_exp(t, e):
    # which exp tiles go to VectorE (rest on ScalarE). Skip the first two t of
    # each head-pair (DVE still draining the previous pair's evac/divides).
    return DVE_EXP and t >= 2 and (2 * t + e) % 6 < 2


def _chunks(n):
    out, ofs = [], 0
    while ofs < n:
        c = min(512, n - ofs)
        out.append((ofs, c))
        ofs += c
    return out


def build_nc(sim_compat=False):
    nc = bacc.Bacc("TRN2", target_bir_lowering=False, debug=False, num_devices=8)

    lat = nc.dram_tensor("lat", [S, D], F32, kind="ExternalInput").ap()
    wq = nc.dram_tensor("wq", [D, D], BF16, kind="ExternalInput").ap()
    wk = nc.dram_tensor("wk", [D, D], BF16, kind="ExternalInput").ap()
    wv = nc.dram_tensor("wv", [D, D], BF16, kind="ExternalInput").ap()
    wo = nc.dram_tensor("wo", [D, D], BF16, kind="ExternalInput").ap()
    wi = nc.dram_tensor("wi", [D, MLP], BF16, kind="ExternalInput").ap()
    wom = nc.dram_tensor("wom", [MLP, D], BF16, kind="ExternalInput").ap()
    kqsc = nc.dram_tensor("kqsc", [128, 1], F32, kind="ExternalInput").ap()
    out = nc.dram_tensor("out", [SQ, D], F32, kind="ExternalOutput").ap()

    with tile.TileContext(nc) as tc, ExitStack() as top:
        def ptile(pool, shape, dtype, name):
            return pool.tile(shape, dtype, name=name, tag=name)

        p_const = top.enter_context(tc.tile_pool(name="p_const", bufs=1))
        p_x2 = top.enter_context(tc.tile_pool(name="p_x2", bufs=1))
        p_oT = tc.alloc_tile_pool(name="p_oT", bufs=1)
        p_att = tc.alloc_tile_pool(name="p_att", bufs=1)

        # ---- persistent tiles ----
        Vaug = ptile(p_att, [128, NT_S * H * VW], BF16, name="Vaug")
        oT = ptile(p_oT, [128, NT_D * SQ], BF16, name="oT")
        kqsc_t = ptile(p_const, [128, 1], F32, name="kqsc_t")
        onesF = ptile(p_const, [128, 64], F32, name="onesF")
        eps_t = ptile(p_const, [128, 1], F32, name="eps_t")
        rs_all = ptile(p_const, [128, NT_S], F32, name="rs_all")
        KT = [ptile(p_att, [128, S], BF16, name=f"KT{d}") for d in range(NT_D)]
        QT = [ptile(p_att, [128, SQ], BF16, name=f"QT{d}") for d in range(NT_D)]
        x2 = [ptile(p_x2, [128, D], F32, name=f"x2_{q}") for q in range(NT_Q)]
        x2T = [ptile(p_x2, [128, SQ], BF16, name=f"x2T{d}") for d in range(NT_D)]

        nc.sync.dma_start(kqsc_t[:], kqsc[:])
        nc.vector.memset(eps_t[:], EPS)
        nc.vector.memset(onesF[:], 1.0)
        vview = Vaug[:].rearrange("p (s h k) -> p s h k", s=NT_S, h=H)
        nc.vector.memset(vview[:, :, :, HD:VW], 1.0)

        dram = top.enter_context(tc.tile_pool(name="dram", bufs=1, space="DRAM"))
        xh_d = [dram.tile([CH, D], BF16, name=f"xh_d{c}") for c in range(NC_S)]
        kh_d = [dram.tile([CH, D], BF16, name=f"kh_d{c}") for c in range(NC_S)]
        qh_d = [dram.tile([CH, D], BF16, name=f"qh_d{c}") for c in range(NC_Q)]
        x2h_d = [dram.tile([CH, D], BF16, name=f"x2h_d{c}") for c in range(NC_Q)]

        # QKV weights: prefetch on the scalar HWDGE queue so the sync queue
        # stays free for the Phase A latent loads.
        p_wqkv = tc.alloc_tile_pool(name="p_wqkv", bufs=1, side="right")
        wq_sb = [ptile(p_wqkv, [128, D], BF16, name=f"wq_sb{d}") for d in range(NT_D)]
        wk_sb = [ptile(p_wqkv, [128, D], BF16, name=f"wk_sb{d}") for d in range(NT_D)]
        wv_sb = [ptile(p_wqkv, [128, D], BF16, name=f"wv_sb{d}") for d in range(NT_D)]
        def _load_qkv_weights(group):
            for w_sb, w_d in group:
                for d in range(NT_D):
                    nc.sync.dma_start(w_sb[d][:], w_d[d * 128:(d + 1) * 128, :])

        # =============== Phase A: ln1 stats + raw-x bf16 + x^T ===============
        # qk-rmsnorm is invariant to the per-row ln1 scaling, so Q/K project
        # the raw latents; only V needs the 1/rms factor (applied per-partition
        # in natural layout in Phase B). x^T tiles hold RAW bf16 latents.
        p_xT = tc.alloc_tile_pool(name="p_xT", bufs=1, side="right")
        xT = [ptile(p_xT, [128, S], BF16, name=f"xT{d}") for d in range(NT_D)]
        with ExitStack() as ctx:
            io = ctx.enter_context(tc.tile_pool(name="a_io", bufs=8))
            st_p = ctx.enter_context(tc.tile_pool(name="a_stats", bufs=8))
            scr = ctx.enter_context(tc.tile_pool(name="a_scr", bufs=4))
            # Everything bulk rides the sync queue, grouped so the chunk
            # transposes interleave between load groups (no head-of-line
            # blocking) and the compute engines carry zero dma_start cost
            # (a 128-partition dma_start costs ~1-2us of issuing-sequencer
            # time -- on scalar it delays the whole ln1 stats chain).
            lts = []

            def _load_group(g):
                for t in range(4 * g, 4 * g + 4):
                    lt = io.tile([128, D], F32, name="lt")
                    nc.sync.dma_start(lt[:], lat[t * 128:(t + 1) * 128, :])
                    lts.append(lt)
                if g == 0:
                    _load_qkv_weights(((wk_sb, wk),))
                elif g == 1:
                    _load_qkv_weights(((wv_sb, wv), (wq_sb, wq)))

            _load_group(0)
            for t in range(NT_S):
                if t % 4 == 0 and t // 4 + 1 < NC_S:
                    _load_group(t // 4 + 1)
                lt = lts[t]
                xh = scr.tile([128, D], BF16, name="xh")
                nc.vector.tensor_copy(xh[:], lt[:])
                r = (t % 4) * 128
                nc.gpsimd.dma_start(xh_d[t // 4][r:r + 128, :], xh[:])
                sq = scr.tile([128, D], F32, name="sq")
                ssq = st_p.tile([128, 1], F32, name="ssq")
                nc.scalar.activation(sq[:], lt[:], AF.Square, accum_out=ssq[:])
                srt = st_p.tile([128, 1], F32, name="srt")
                nc.scalar.activation(srt[:], ssq[:], AF.Sqrt, bias=eps_t[:], scale=1.0 / D)
                nc.vector.reciprocal(rs_all[:, t:t + 1], srt[:])
                if t % 4 == 3:  # chunk of 512 rows complete -> transpose it
                    c = t // 4
                    for d in range(NT_D):
                        nc.sync.dma_start_transpose(
                            xT[d][:, c * CH:(c + 1) * CH],
                            xh_d[c][:, d * 128:(d + 1) * 128])

        # =============== Phase B: Q/K/V projections + qk-norm ===============
        with ExitStack() as ctx:
            ps = ctx.enter_context(tc.tile_pool(name="b_ps", bufs=4, space="PSUM"))
            scr = ctx.enter_context(tc.tile_pool(name="b_scr", bufs=3))
            st_p = ctx.enter_context(tc.tile_pool(name="b_stats", bufs=6))
            natp = ctx.enter_context(tc.tile_pool(name="b_nat", bufs=3))

            def proj(t, w_sb):
                p = ps.tile([128, D], F32, name="p_proj")
                for d in range(NT_D):
                    lhsT = xT[d][:, t * 128:(t + 1) * 128]
                    for ofs, n in _chunks(D):
                        nc.tensor.matmul(
                            p[:, ofs:ofs + n], lhsT, w_sb[d][:, ofs:ofs + n],
                            start=(d == 0), stop=(d == NT_D - 1))
                return p

            def qknorm(p, dst_dram, t):
                sq = scr.tile([128, D], F32, name="sq_b")
                nc.scalar.activation(sq[:], p[:], AF.Square)
                ss = st_p.tile([128, H], F32, name="ss_b")
                nc.vector.tensor_reduce(
                    ss[:], sq[:].rearrange("p (h k) -> p h k", h=H),
                    axis=mybir.AxisListType.X, op=OP.add)
                srt = st_p.tile([128, H], F32, name="srt_b")
                nc.scalar.activation(srt[:], ss[:], AF.Sqrt, bias=eps_t[:], scale=1.0 / HD)
                rs = st_p.tile([128, H], F32, name="rs_b")
                nc.vector.reciprocal(rs[:], srt[:])
                nat = natp.tile([128, D], BF16, name="nat_b")
                rs_view = rs[:].rearrange("p (h o) -> p h o", o=1).broadcast_to([128, H, HD])
                nc.vector.tensor_tensor(
                    out=nat[:].rearrange("p (h k) -> p h k", h=H),
                    in0=p[:].rearrange("p (h k) -> p h k", h=H),
                    in1=rs_view, op=OP.mult)
                r = (t % 4) * 128
                nc.gpsimd.dma_start(dst_dram[t // 4][r:r + 128, :], nat[:])

            for t in range(NT_S):
                pk = proj(t, wk_sb)
                qknorm(pk, kh_d, t)
                pv = proj(t, wv_sb)
                nc.vector.tensor_scalar_mul(
                    vview[:, t, :, 0:HD],
                    pv[:].rearrange("p (h k) -> p h k", h=H),
                    rs_all[:, t:t + 1])
                if t < NT_Q:
                    pq = proj(t, wq_sb)
                    qknorm(pq, qh_d, t)
                if t % 4 == 3:  # pipeline K^T (and Q^T) transposes per 512-row chunk
                    c = t // 4
                    for d in range(NT_D):
                        nc.sync.dma_start_transpose(
                            KT[d][:, c * CH:(c + 1) * CH],
                            kh_d[c][:, d * 128:(d + 1) * 128])
                        if c < NC_Q:
                            # kqsc (qk-norm scales / sqrt(hd)) folded into Q^T
                            # only: logits contract q*k over hd, so one side
                            # suffices (12 scale ops instead of 24 on K^T).
                            nc.sync.dma_start_transpose(
                                QT[d][:, c * CH:(c + 1) * CH],
                                qh_d[c][:, d * 128:(d + 1) * 128])
            # QT kqsc scales batched here: the transposes are complete, so
            # these don't head-of-line block the DVE queue mid-loop.
            for d in range(NT_D):
                nc.vector.tensor_scalar_mul(QT[d][:], QT[d][:], kqsc_t[:])
        p_xT.release()
        p_wqkv.release()

        # MLP1 + attn-out weights: prefetch during attention (sync queue idle)
        p_wi = tc.alloc_tile_pool(name="p_wi", bufs=1, side="right")
        wi_sb = [ptile(p_wi, [128, MLP], BF16, name=f"wi_sb{d}") for d in range(NT_D)]
        p_wo = tc.alloc_tile_pool(name="p_wo", bufs=1, side="right")
        wo_sb = [ptile(p_wo, [128, D], BF16, name=f"wo_sb{d}") for d in range(NT_D)]
        for d in range(NT_D):
            nc.sync.dma_start(wo_sb[d][:], wo[d * 128:(d + 1) * 128, :])
            nc.sync.dma_start(wi_sb[d][:], wi[d * 128:(d + 1) * 128, :])

        # =============== Phase C: attention ===============
        with ExitStack() as ctx:
            psL = ctx.enter_context(tc.tile_pool(name="c_psL", bufs=2, space="PSUM"))
            psO = ctx.enter_context(tc.tile_pool(name="c_psO", bufs=2, space="PSUM"))
            pp = ctx.enter_context(tc.tile_pool(name="c_p", bufs=6))
            oup = ctx.enter_context(tc.tile_pool(name="c_oU", bufs=5))

            def divide_head(h, oU_h, den_h, row):
                # broadcast 1/denom (at partition `row` of den_h) across 64
                # partitions via ones outer product, then multiply.
                dt, base = h // 2, (h % 2) * 64
                b_ps = psL.tile([64, SQ], F32, name="b_ps", tag="l_ps")
                for ofs, n in _chunks(SQ):
                    nc.tensor.matmul(b_ps[:, ofs:ofs + n], onesF[row:row + 1, :],
                                     den_h[row:row + 1, ofs:ofs + n],
                                     start=True, stop=True)
                nc.vector.scalar_tensor_tensor(
                    oT[base:base + 64, dt * SQ:(dt + 1) * SQ],
                    b_ps[:], 1.0, oU_h[0:HD, :], op0=OP.bypass, op1=OP.mult)

            def attn_v(hp, t, o_ps, p_t):
                for e in range(2):
                    vofs = t * H * VW + (2 * hp + e) * VW
                    for ofs, n in _chunks(SQ):
                        nc.tensor.matmul(
                            o_ps[e][:, ofs:ofs + n],
                            Vaug[:, vofs:vofs + VW],
                            p_t[e][:].bitcast(BF16)[:, ofs:ofs + n],
                            start=(t == 0), stop=(t == NT_S - 1))

            pending = []
            for hp in range(H // 2):
                dt = hp
                o_ps = [psO.tile([VW, SQ], F32, name=f"o_ps{e}", tag="o_ps")
                        for e in range(2)]
                prev = None  # software pipeline: attnV runs one t behind the
                # logits/exp of t, so the exp has a full extra unit of slack
                # and the PE stream stays gap-free (keeps HAM at full clock).
                for t in range(NT_S):
                    l_ps = [psL.tile([128, SQ], F32, name=f"l_ps{e}", tag="l_ps")
                            for e in range(2)]
                    for e in range(2):  # head 2*hp+e at PE row group 64*e
                        base = 64 * e
                        lhsT = KT[dt][base:base + 64, t * 128:(t + 1) * 128]
                        for j in range(NC_Q):
                            nc.tensor.matmul(
                                l_ps[e][:, j * CH:(j + 1) * CH], lhsT,
                                QT[dt][base:base + 64, j * CH:(j + 1) * CH],
                                start=True, stop=True)
                    # attnV of t-1 emitted before the exps of t so the PE
                    # queue keeps the two logits pairs adjacent (row-group
                    # concurrency) and attnV consumes the already-finished exp.
                    if prev is not None:
                        attn_v(hp, prev[0], o_ps, prev[1])
                    p_t = [None, None]
                    for e in range(2):
                        p_t[e] = pp.tile([128, SQ], I16, name=f"p_t{e}", tag="p_t")
                        if _dve_exp(t, e):
                            nc.vector.tensor_scalar(
                                p_t[e][:], l_ps[e][:], A_EXP, B_EXP,
                                op0=OP.mult, op1=OP.add)
                        else:
                            nc.scalar.activation(
                                p_t[e][:].bitcast(BF16), l_ps[e][:], AF.Exp)
                    prev = (t, p_t)
                    if t == 4:
                        for h_prev, oU_prev, den_prev, row in pending:
                            divide_head(h_prev, oU_prev, den_prev, row)
                        pending = []
                attn_v(hp, prev[0], o_ps, prev[1])
                # denominator rows of both heads -> partitions 0/32 of one
                # scratch (custom-DVE ops only work at partition offset 0),
                # one batched reciprocal for the pair.
                den_t = oup.tile([33, SQ], F32, name="den", tag="den")
                oUs = []
                for e in range(2):
                    oU_h = oup.tile([VW, SQ], F32, name="oU", tag="oU")
                    nc.vector.tensor_copy(oU_h[:], o_ps[e][:])
                    nc.sync.dma_start(den_t[32 * e:32 * e + 1, :],
                                      oU_h[VW - 1:VW, :])
                    oUs.append(oU_h)
                if FAST_RECIP:
                    nc.vector.reciprocal_approx_fast(den_t[:], den_t[:])
                else:
                    nc.vector.reciprocal(den_t[0:1, :], den_t[0:1, :])
                    nc.vector.reciprocal(den_t[32:33, :], den_t[32:33, :])
                for e in range(2):
                    pending.append((2 * hp + e, oUs[e], den_t, 32 * e))
            for h_prev, oU_prev, den_prev, row in pending:
                divide_head(h_prev, oU_prev, den_prev, row)
        p_att.release()

        p_wom = tc.alloc_tile_pool(name="p_wom", bufs=1, side="right")
        wom_sb = [ptile(p_wom, [128, D], BF16, name=f"wom_sb{m}") for m in range(NT_M)]

        # =============== Phase D: out-proj + residual + ln2 ===============
        with ExitStack() as ctx:
            ps = ctx.enter_context(tc.tile_pool(name="d_ps", bufs=2, space="PSUM"))
            io = ctx.enter_context(tc.tile_pool(name="d_io", bufs=NT_Q))
            scr = ctx.enter_context(tc.tile_pool(name="d_scr", bufs=3))
            st_p = ctx.enter_context(tc.tile_pool(name="d_stats", bufs=4))

            lts = []
            for q in range(NT_Q):
                lt = io.tile([128, D], F32, name="lt_d")
                nc.sync.dma_start(lt[:], lat[q * 128:(q + 1) * 128, :])
                lts.append(lt)
            for q in range(NT_Q):
                p = ps.tile([128, D], F32, name="p_oproj")
                for d in range(NT_D):
                    for ofs, n in _chunks(D):
                        nc.tensor.matmul(
                            p[:, ofs:ofs + n],
                            oT[:, d * SQ + q * 128: d * SQ + (q + 1) * 128],
                            wo_sb[d][:, ofs:ofs + n],
                            start=(d == 0), stop=(d == NT_D - 1))
                nc.vector.tensor_tensor(out=x2[q][:], in0=p[:], in1=lts[q][:], op=OP.add)
                sq = scr.tile([128, D], F32, name="sq_d")
                ssq = st_p.tile([128, 1], F32, name="ssq_d")
                nc.scalar.activation(sq[:], x2[q][:], AF.Square, accum_out=ssq[:])
                srt = st_p.tile([128, 1], F32, name="srt_d")
                nc.scalar.activation(srt[:], ssq[:], AF.Sqrt, bias=eps_t[:], scale=1.0 / D)
                rs = st_p.tile([128, 1], F32, name="rs_d")
                nc.vector.reciprocal(rs[:], srt[:])
                xh2 = scr.tile([128, D], BF16, name="xh2")
                nc.vector.tensor_scalar_mul(xh2[:], x2[q][:], rs[:])
                r = (q % 4) * 128
                nc.gpsimd.dma_start(x2h_d[q // 4][r:r + 128, :], xh2[:])
                if q % 4 == 3:  # pipeline x2^T transposes per 512-row chunk
                    c = q // 4
                    for d in range(NT_D):
                        nc.sync.dma_start_transpose(
                            x2T[d][:, c * CH:(c + 1) * CH],
                            x2h_d[c][:, d * 128:(d + 1) * 128])
            # MLP2 weights after the transposes: needed only by MLP2 (~60us
            # away); keeps their dma_start cost off the compute engines.
            for m in range(NT_M):
                nc.sync.dma_start(wom_sb[m][:], wom[m * 128:(m + 1) * 128, :])
        p_oT.release()

        # =============== Phase E: MLP ===============
        p_hT = tc.alloc_tile_pool(name="p_hT", bufs=1, side="right")
        hT = ptile(p_hT, [128, NT_M * SQ], BF16, name="hT")
        with ExitStack() as ctx:
            ps = ctx.enter_context(tc.tile_pool(name="e_ps", bufs=1, space="PSUM"))
            iop = ctx.enter_context(tc.tile_pool(name="e_io", bufs=3))

            for m in range(NT_M):
                p = ps.tile([128, SQ], F32, name="p_mlp1", bufs=2)
                # j-outer: the j=0 half only needs x2T chunk 0 (first half of
                # Phase D) -> PE starts MLP1 while D finishes.
                for j in range(NC_Q):
                    for d in range(NT_D):
                        nc.tensor.matmul(
                            p[:, j * CH:(j + 1) * CH],
                            wi_sb[d][:, m * 128:(m + 1) * 128],
                            x2T[d][:, j * CH:(j + 1) * CH],
                            start=(d == 0), stop=(d == NT_D - 1))
                if not sim_compat:
                    nc.scalar.activation(hT[:, m * SQ:(m + 1) * SQ], p[:],
                                         AF.Gelu_apprx_tanh)
                else:
                    xsq = iop.tile([128, SQ], F32, name="g_xsq", bufs=1)
                    nc.vector.tensor_tensor(out=xsq[:], in0=p[:], in1=p[:], op=OP.mult)
                    w = iop.tile([128, SQ], F32, name="g_w", bufs=1)
                    nc.vector.tensor_scalar(w[:], xsq[:], 0.044715, 1.0,
                                            op0=OP.mult, op1=OP.add)
                    u = iop.tile([128, SQ], F32, name="g_u", bufs=1)
                    nc.vector.tensor_tensor(out=u[:], in0=w[:], in1=p[:], op=OP.mult)
                    th = iop.tile([128, SQ], F32, name="g_th", bufs=1)
                    nc.scalar.activation(th[:], u[:], AF.Tanh, scale=0.7978845608028654)
                    t2 = iop.tile([128, SQ], F32, name="g_t2", bufs=1)
                    nc.vector.scalar_tensor_tensor(t2[:], th[:], 1.0, p[:],
                                                   op0=OP.add, op1=OP.mult)
                    nc.vector.tensor_scalar_mul(hT[:, m * SQ:(m + 1) * SQ], t2[:], 0.5)

            for q in range(NT_Q):
                p = ps.tile([128, D], F32, name="p_mlp2", bufs=2)
                for m in range(NT_M):
                    for ofs, n in _chunks(D):
                        nc.tensor.matmul(
                            p[:, ofs:ofs + n],
                            hT[:, m * SQ + q * 128: m * SQ + (q + 1) * 128],
                            wom_sb[m][:, ofs:ofs + n],
                            start=(m == 0), stop=(m == NT_M - 1))
                ot = iop.tile([128, D], F32, name="ot_e")
                nc.vector.tensor_tensor(out=ot[:], in0=p[:], in1=x2[q][:], op=OP.add)
                nc.sync.dma_start(out[q * 128:(q + 1) * 128, :], ot[:])
        p_hT.release()
        p_wom.release()
        p_wo.release()
        p_wi.release()

    nc.compile()
    return nc


def make_in_maps(latents, ln1_scale, wq, wk, wv, q_norm_scale, k_norm_scale,
                 wo_attn, ln2_scale, wi, wo_mlp):
    import ml_dtypes
    bf = ml_dtypes.bfloat16
    wq2 = (np.asarray(ln1_scale, np.float64)[:, None]
           * np.asarray(wq, np.float64).reshape(D, D)).astype(bf)
    wk2 = (np.asarray(ln1_scale, np.float64)[:, None]
           * np.asarray(wk, np.float64).reshape(D, D)).astype(bf)
    wv2 = (np.asarray(ln1_scale, np.float64)[:, None]
           * np.asarray(wv, np.float64).reshape(D, D)).astype(bf)
    wo2 = np.asarray(wo_attn, np.float32).reshape(D, D).astype(bf)
    wi2 = (np.asarray(ln2_scale, np.float64)[:, None]
           * np.asarray(wi, np.float64)).astype(bf)
    wom2 = np.asarray(wo_mlp, np.float32).astype(bf)
    kq = (np.tile(np.asarray(q_norm_scale, np.float64)
                  * np.asarray(k_norm_scale, np.float64), 2)
          / np.sqrt(HD)).astype(np.float32)[:, None]
    lat_np = np.asarray(latents, np.float32)
    in_maps = []
    for c in range(8):
        b, half = c // 2, c % 2
        lm = lat_np[b]
        lat_rot = np.concatenate([lm[half * SQ:(half + 1) * SQ],
                                  lm[(1 - half) * SQ:(2 - half) * SQ]], axis=0)
        in_maps.append(dict(lat=np.ascontiguousarray(lat_rot), wq=wq2, wk=wk2,
                            wv=wv2, wo=wo2, wi=wi2, wom=wom2, kqsc=kq))
    return in_maps


_NC_CACHE = None


def kernel(**inputs):
    global _NC_CACHE
    if _NC_CACHE is None:
        _NC_CACHE = build_nc()
    nc = _NC_CACHE
    in_maps = make_in_maps(**inputs)
    res = run_bass_kernel_spmd(nc, in_maps, list(range(8)))
    y = np.empty((B, S, D), np.float32)
    for c in range(8):
        b, half = c // 2, c % 2
        y[b, half * SQ:(half + 1) * SQ] = res.results[c]["out"]
    return y


if __name__ == "__main__":
    import reference
    inputs = {k: np.asarray(v) for k, v in reference.setup_inputs().items()}
    y = kernel(**inputs)
    exp = np.asarray(reference.reference(**reference.setup_inputs()))
    err = np.abs(y - exp).max() / np.abs(exp).max()
    print("Relative error:", err)
